# revision 14
# baseline (speedup 1.0000x reference)
"""Coordinate-descent (alternating Gauss-Seidel) kernel for Trainium2, v5.

B=4 factorizations x ~ u @ v^T, M=N=4096, R=32.
The per-column GS sweep is algebraically a triangular solve:
    u_new = (a + eps - u @ B_sl) @ M^{-1},   M = diag(B)+eps + triu(B,1)
with B = v^T v.  M^{-1} is applied exactly via the nilpotent factorization
    (I+W)^{-1} = (I-W)(I+W^2)(I+W^4)(I+W^8)(I+W^16),  W = triu(B,1) D'^{-1}
so each half-step is a handful of PE matmuls instead of a 32-step
vector-engine recurrence.  All work stays in transposed [R, m] space.

The u-side Grams b1 = v^T v and their W-chains depend only on v, so they are
hoisted to kernel start (also serving as HAM warm-up).  Phase-2 partials
(a2T = u_new^T x, b2) use 4-way col-tiled matmuls and one fused per-batch
ReduceScatter; the v-solve of batch b is emitted two batches later so no
engine queue ever stalls on a collective.
"""

import os
from contextlib import ExitStack

import numpy as np

import concourse.bass as bass
import concourse.tile as tile
from concourse import bacc, mybir
from concourse.bass import ds
from concourse.bass_utils import run_bass_kernel_spmd
from concourse.masks import make_identity, make_lower_triangular

B, M, N, R = 4, 4096, 4096, 32
NCORES = 8
MS = M // NCORES          # 512 rows per core per batch
MC = MS // 128            # 4 m-chunks of 128
NG = N // 512             # 8 n-groups of 512
NCH = N // 128            # 32 n-chunks of 128
EPS = 1e-8
F32 = mybir.dt.float32
F32R = mybir.dt.float32r
BF16 = mybir.dt.bfloat16
ALU = mybir.AluOpType
AX = mybir.AxisListType

_CACHE = {}
LAST_RESULT = None


def _gram_prep(nc, smp, pwp, punp, consts, b_sb, tg):
    """Precompute inv_p, B_sl and the W-chain from a Gram matrix (SBUF)."""
    ident32_r, masksl_r, eye_r = consts

    bd = smp.tile([R, R], F32R, tag=f"bd{tg}", name="bd", bufs=1)
    nc.vector.tensor_tensor(out=bd[:], in0=b_sb[:], in1=eye_r, op=ALU.mult)
    d_p = smp.tile([R, 1], F32, tag=f"dp{tg}", name="d_p", bufs=1)
    inv_p = smp.tile([R, 1], F32, tag=f"ip{tg}", name="inv_p", bufs=1)
    nc.vector.tensor_reduce(d_p[:], bd[:], axis=AX.X, op=ALU.add)
    nc.vector.tensor_scalar_add(inv_p[:], d_p[:], EPS)
    nc.vector.reciprocal(inv_p[:], inv_p[:])
    invb = bass.AP(inv_p[:].tensor, inv_p[:].offset, [inv_p[:].ap[0], [0, R]])

    bsl = smp.tile([R, R], F32R, tag=f"bsl{tg}", name="bsl", bufs=1)
    nc.vector.tensor_tensor(out=bsl[:], in0=b_sb[:], in1=masksl_r,
                            op=ALU.mult)
    vw = smp.tile([R, R], F32R, tag=f"vw{tg}", name="vw", bufs=1)  # V = W^T
    nc.vector.tensor_tensor(out=vw[:], in0=bsl[:], in1=invb, op=ALU.mult)

    # transpose W^T -> W (f32r out must live in the f32r pun slot)
    pwt = punp.tile([128, MC, R], F32R, tag="pun", name="pwt")
    nc.tensor.transpose(pwt[:R, 0, :], vw[:], ident32_r)
    w1 = smp.tile([R, R], F32R, tag=f"w1{tg}", name="w1", bufs=1)
    nc.scalar.copy(w1[:], pwt[:R, 0, :])

    def _mm_small(lhsT, rhs, tagn):
        p = pwp.tile([R, R], F32, tag="pw", name="pmm")
        nc.tensor.matmul(p[:], lhsT=lhsT[:], rhs=rhs[:], start=True,
                         stop=True)
        s = smp.tile([R, R], F32R, tag=f"{tagn}{tg}", name=tagn, bufs=1)
        nc.scalar.copy(s[:], p[:])
        return s

    w2 = _mm_small(vw, w1, "w2")     # W^T.T @ W = W@W
    w2t = _mm_small(w1, vw, "w2t")   # W.T @ W^T = (W@W)^T
    w4 = _mm_small(w2t, w2, "w4")
    w4t = _mm_small(w2, w2t, "w4t")
    w8 = _mm_small(w4t, w4, "w8")
    w8t = _mm_small(w4, w4t, "w8t")
    w16 = _mm_small(w8t, w8, "w16")
    return {"bsl": bsl, "inv_p": inv_p, "chain": (w1, w2, w4, w8, w16)}


def _apply_solve(nc, smp, zsb, punp, zps, consts, prep, at_sb, xT_sb, out32,
                 outb16, tg):
    """sT = B_sl^T @ xT; z = P^T D'^{-1} (at - sT); back-transpose."""
    ident32_r, _, _ = consts
    bsl, inv_p = prep["bsl"], prep["inv_p"]

    ps = zps("s")
    nc.tensor.matmul(ps[:], lhsT=bsl[:], rhs=xT_sb[:], start=True, stop=True)
    z = zsb.tile([R, MS], F32R, tag=f"z{tg}", name="z0")
    nc.vector.scalar_tensor_tensor(out=z[:], in0=ps[:], scalar=-1.0,
                                   in1=at_sb[:], op0=ALU.mult, op1=ALU.add)
    nc.vector.tensor_scalar(out=z[:], in0=z[:], scalar1=inv_p[:],
                            scalar2=None, op0=ALU.mult)

    for wk, sign in zip(prep["chain"], (-1.0, 1.0, 1.0, 1.0, 1.0)):
        pz = zps("z")
        nc.tensor.matmul(pz[:], lhsT=wk[:], rhs=z[:], start=True, stop=True)
        zn = zsb.tile([R, MS], F32R, tag=f"z{tg}", name="zn")
        nc.vector.scalar_tensor_tensor(out=zn[:], in0=pz[:], scalar=sign,
                                       in1=z[:], op0=ALU.mult, op1=ALU.add)
        z = zn

    pun = punp.tile([128, MC, R], F32R, tag="pun", name="pun")
    for i in range(MC):
        nc.tensor.transpose(pun[:, i], z[:, i * 128:(i + 1) * 128],
                            ident32_r)
    nc.scalar.copy(out32[:], pun[:])
    if outb16 is not None:
        nc.vector.tensor_copy(outb16[:], pun[:])


def _build():
    nc = bacc.Bacc("TRN2", target_bir_lowering=False, debug=False,
                   num_devices=NCORES)

    x_my = nc.dram_tensor("x_my", [B, MS, N], F32, kind="ExternalInput").ap()
    u_my = nc.dram_tensor("u_my", [B, MS, R], F32, kind="ExternalInput").ap()
    v_full = nc.dram_tensor("v_full", [B, N, R], F32,
                            kind="ExternalInput").ap()
    v_my = nc.dram_tensor("v_my", [B, MS, R], F32, kind="ExternalInput").ap()
    u_out = nc.dram_tensor("u_out", [B, MS, R], F32,
                           kind="ExternalOutput").ap()
    v_out = nc.dram_tensor("v_out", [B, MS, R], F32,
                           kind="ExternalOutput").ap()

    rs_ins = [nc.dram_tensor(f"rs_in_{b}", [NCORES * R, 512 + R], F32)
              for b in range(B)]
    rs_outs = [nc.dram_tensor(f"rs_out_{b}", [R, 512 + R], F32)
               for b in range(B)]

    with tile.TileContext(nc) as tc, ExitStack() as ctx:
        const = ctx.enter_context(tc.tile_pool(name="const", bufs=1))
        xbp = ctx.enter_context(tc.tile_pool(name="xbp", bufs=1))
        xgp = ctx.enter_context(tc.tile_pool(name="xgp", bufs=3))
        xtp = ctx.enter_context(tc.tile_pool(name="xtp", bufs=4))
        vp = ctx.enter_context(tc.tile_pool(name="vp", bufs=2))
        smp = ctx.enter_context(tc.tile_pool(name="smp", bufs=2))
        zsb = ctx.enter_context(tc.tile_pool(name="zsb", bufs=3))
        a2sp = ctx.enter_context(tc.tile_pool(name="a2sp", bufs=2))
        # PSUM banks: ppt 2 + pa1 1 + pzu 1 + pw 1 + pun 1 + pa2 1 + psv 1 = 8
        ppt = ctx.enter_context(tc.tile_pool(name="ppt", bufs=2,
                                             space="PSUM"))
        pa1p = ctx.enter_context(tc.tile_pool(name="pa1", bufs=1,
                                              space="PSUM"))
        pzup = ctx.enter_context(tc.tile_pool(name="pzu", bufs=1,
                                              space="PSUM"))
        pwp = ctx.enter_context(tc.tile_pool(name="pw", bufs=1,
                                             space="PSUM"))
        punp = ctx.enter_context(tc.tile_pool(name="pun", bufs=1,
                                              space="PSUM"))
        pa2p = ctx.enter_context(tc.tile_pool(name="pa2", bufs=1,
                                              space="PSUM"))
        psvp = ctx.enter_context(tc.tile_pool(name="psv", bufs=1,
                                              space="PSUM"))

        ident128_b = const.tile([128, 128], BF16)
        make_identity(nc, ident128_b)
        ident128_f = const.tile([128, 128], F32)
        make_identity(nc, ident128_f)
        ident32_f = const.tile([R, R], F32)
        make_identity(nc, ident32_f)
        masksl_f = const.tile([R, R], F32)
        make_lower_triangular(nc, masksl_f, val=1.0, diag=False)
        ident32_r = const.tile([R, R], F32R)
        nc.vector.tensor_copy(ident32_r[:], ident32_f[:])
        masksl_r = const.tile([R, R], F32R)
        nc.vector.tensor_copy(masksl_r[:], masksl_f[:])
        consts = (ident32_r[:], masksl_r[:], ident32_r[:])

        def zps(nm):
            return pzup.tile([R, MS], F32, tag="zu", name=nm)

        def zps_v(nm):
            return psvp.tile([R, MS], F32, tag="zv", name=nm)

        # ---------- hoisted: v loads, b1 Grams, u-side W-chains ----------
        vbs = []
        u_preps = []
        for b in range(B):
            v32 = vp.tile([128, NCH, R], F32, tag="v32", name="v32")
            nc.sync.dma_start(v32[:],
                              v_full[b].rearrange("(c p) r -> p c r", p=128))
            vb = vp.tile([128, NCH, R], BF16, tag=f"vb{b}", name="vb",
                         bufs=1)
            nc.vector.tensor_copy(vb[:], v32[:])
            vbs.append(vb)

            pb1 = zps("pb1")
            for j in range(NCH):
                nc.tensor.matmul(pb1[:, :R], lhsT=vb[:, j, :],
                                 rhs=vb[:, j, :], start=(j == 0),
                                 stop=(j == NCH - 1), skip_group_check=True)
            b1_sb = smp.tile([R, R], F32R, tag=f"b1s{b}", name="b1_sb",
                             bufs=1)
            nc.scalar.copy(b1_sb[:], pb1[:, :R])
            u_preps.append(_gram_prep(nc, smp, pwp, punp, consts, b1_sb,
                                      f"u{b}"))

        state = {}

        def emit_solve_v(b):
            st = state[b]
            a2t = smp.tile([R, MS], F32R, tag="a2t", name="a2t")
            nc.sync.dma_start(a2t[:],
                              rs_outs[b].ap()[:, 0:512].bitcast(F32R))
            b2_sb = smp.tile([R, R], F32R, tag="b2s", name="b2_sb")
            nc.sync.dma_start(b2_sb[:],
                              rs_outs[b].ap()[:, 512:512 + R].bitcast(F32R))
            at2 = smp.tile([R, MS], F32R, tag="at2", name="at2")
            nc.vector.tensor_scalar_add(at2[:], a2t[:], EPS)
            prep = _gram_prep(nc, smp, pwp, punp, consts, b2_sb, f"v{b}")
            vn32 = smp.tile([128, MC, R], F32, tag="vn32", name="vn32")
            _apply_solve(nc, smp, zsb, punp, zps_v, consts, prep, at2,
                         st["vT"], vn32, None, "v")
            nc.sync.dma_start(v_out[b].rearrange("(i p) r -> p i r", p=128),
                              vn32[:])

        for b in range(B):
            bi = b % 2
            # ---------------- per-batch loads + uT/vT ----------------
            u32 = vp.tile([128, MC, R], F32, tag="u32", name="u32")
            nc.sync.dma_start(u32[:],
                              u_my[b].rearrange("(i p) r -> p i r", p=128))
            vm32 = vp.tile([128, MC, R], F32, tag="vm32", name="vm32")
            nc.sync.dma_start(vm32[:],
                              v_my[b].rearrange("(i p) r -> p i r", p=128))

            put = zps("put")
            for i in range(MC):
                nc.tensor.transpose(put[:, i * 128:(i + 1) * 128],
                                    u32[:, i, :], ident128_f[:])
            uT = smp.tile([R, MS], F32R, tag="uT", name="uT")
            nc.scalar.copy(uT[:], put[:])
            pvt = zps("pvt")
            for i in range(MC):
                nc.tensor.transpose(pvt[:, i * 128:(i + 1) * 128],
                                    vm32[:, i, :], ident128_f[:])
            vT = smp.tile([R, MS], F32R, tag="vT", name="vT", bufs=3)
            nc.scalar.copy(vT[:], pvt[:])

            # ---------------- phase 1: stream x ----------------
            xb_t = xbp.tile([128, MC, N], BF16, tag=f"xb{bi}", name="xb")
            pa1 = pa1p.tile([R, MS], F32, tag="pa1", name="pa1")
            x_re = x_my[b].rearrange("(i p) n -> p i n", p=128)
            for g in range(NG):
                xg = xgp.tile([128, MC, 512], F32, tag="xg", name="xg")
                nc.sync.dma_start(xg[:], x_re[:, :, g * 512:(g + 1) * 512])
                nc.vector.tensor_copy(xb_t[:, :, g * 512:(g + 1) * 512],
                                      xg[:])
                for j2 in range(4):
                    j = 4 * g + j2
                    pt = ppt.tile([128, MC, 128], BF16, tag="pt", name="pt")
                    for i in range(MC):
                        nc.tensor.transpose(
                            pt[:, i], xb_t[:, i, j * 128:(j + 1) * 128],
                            ident128_b[:])
                    xt = xtp.tile([128, MC, 128], BF16, tag="xt", name="xt")
                    nc.scalar.copy(xt[:], pt[:])
                    nc.tensor.matmul(pa1[:], lhsT=vbs[b][:, j, :],
                                     rhs=xt.rearrange("p a b -> p (a b)"),
                                     start=(j == 0), stop=(j == NCH - 1),
                                     skip_group_check=True)

            # ---------------- u solve ----------------
            at1 = smp.tile([R, MS], F32R, tag="at1", name="at1")
            nc.vector.tensor_scalar_add(at1[:], pa1[:], EPS)
            un32 = smp.tile([128, MC, R], F32, tag="un32", name="un32")
            unb = smp.tile([128, MC, R], BF16, tag="unb", name="unb")
            _apply_solve(nc, smp, zsb, punp, zps, consts, u_preps[b], at1,
                         uT, un32, unb, "u")
            nc.sync.dma_start(u_out[b].rearrange("(i p) r -> p i r", p=128),
                              un32[:])

            # ---------------- phase 2 partials + RS ----------------
            for gp in range(2):          # 2 packs of 4 col-tiled n-groups
                pa2 = pa2p.tile([128, MS], F32, tag="pa2", name="pa2")
                for i in range(MC):
                    for p in range(4):
                        g2 = gp * 4 + p
                        nc.tensor.matmul(
                            pa2[p * R:(p + 1) * R, :], lhsT=unb[:, i, :],
                            rhs=xb_t[:, i, g2 * 512:(g2 + 1) * 512],
                            start=(i == 0), stop=(i == MC - 1),
                            tile_position=(0, p * R),
                            skip_group_check=True)
                a2st = a2sp.tile([128, MS], F32, tag="a2st", name="a2st")
                nc.scalar.copy(a2st[:], pa2[:])
                nc.sync.dma_start(rs_ins[b].ap()[ds(gp * 4 * R, 4 * R),
                                                 0:512], a2st[:])
            pb2 = pwp.tile([R, R], F32, tag="pw", name="pb2")
            for i in range(MC):
                nc.tensor.matmul(pb2[:], lhsT=unb[:, i, :], rhs=unb[:, i, :],
                                 start=(i == 0), stop=(i == MC - 1),
                                 skip_group_check=True)
            b2st = a2sp.tile([R, R], F32, tag="b2st", name="b2st")
            nc.scalar.copy(b2st[:], pb2[:])
            for c in range(NCORES):
                nc.sync.dma_start(
                    rs_ins[b].ap()[ds(c * R, R), 512:512 + R], b2st[:])

            nc.gpsimd.collective_compute(
                "ReduceScatter", ALU.add,
                replica_groups=[list(range(NCORES))],
                ins=[rs_ins[b].ap()], outs=[rs_outs[b].ap()])
            state[b] = {"vT": vT}

            # v-solve two batches back (its RS has long completed)
            if b >= 2:
                emit_solve_v(b - 2)

        emit_solve_v(B - 2)
        emit_solve_v(B - 1)

    nc.compile()
    return nc


def kernel(x, u, v):
    global LAST_RESULT
    if "nc" not in _CACHE:
        _CACHE["nc"] = _build()
    nc = _CACHE["nc"]

    x = np.ascontiguousarray(x, dtype=np.float32)
    u = np.ascontiguousarray(u, dtype=np.float32)
    v = np.ascontiguousarray(v, dtype=np.float32)

    in_maps = []
    for c in range(NCORES):
        sl = slice(c * MS, (c + 1) * MS)
        in_maps.append({
            "x_my": np.ascontiguousarray(x[:, sl, :]),
            "u_my": np.ascontiguousarray(u[:, sl, :]),
            "v_full": v,
            "v_my": np.ascontiguousarray(v[:, sl, :]),
        })

    res = run_bass_kernel_spmd(nc, in_maps, list(range(NCORES)),
                               trace=os.environ.get("KBENCH_TRACE") == "1")
    LAST_RESULT = res
    u_new = np.concatenate([res.results[c]["u_out"] for c in range(NCORES)],
                           axis=1)
    v_new = np.concatenate([res.results[c]["v_out"] for c in range(NCORES)],
                           axis=1)
    return (u_new, v_new)


# revision 15
# speedup vs baseline: 1.1297x; 1.1297x over previous
"""Coordinate-descent (alternating Gauss-Seidel) kernel for Trainium2, v5.

B=4 factorizations x ~ u @ v^T, M=N=4096, R=32.
The per-column GS sweep is algebraically a triangular solve:
    u_new = (a + eps - u @ B_sl) @ M^{-1},   M = diag(B)+eps + triu(B,1)
with B = v^T v.  M^{-1} is applied exactly via the nilpotent factorization
    (I+W)^{-1} = (I-W)(I+W^2)(I+W^4)(I+W^8)(I+W^16),  W = triu(B,1) D'^{-1}
so each half-step is a handful of PE matmuls instead of a 32-step
vector-engine recurrence.  All work stays in transposed [R, m] space.

The u-side Grams b1 = v^T v and their W-chains depend only on v, so they are
hoisted to kernel start (also serving as HAM warm-up).  Phase-2 partials
(a2T = u_new^T x, b2) use 4-way col-tiled matmuls and one fused per-batch
ReduceScatter; the v-solve of batch b is emitted two batches later so no
engine queue ever stalls on a collective.
"""

import os
from contextlib import ExitStack

import numpy as np

import concourse.bass as bass
import concourse.tile as tile
from concourse import bacc, mybir
from concourse.bass import ds
from concourse.bass_utils import run_bass_kernel_spmd
from concourse.masks import make_identity, make_lower_triangular

B, M, N, R = 4, 4096, 4096, 32
NCORES = 8
MS = M // NCORES          # 512 rows per core per batch
MC = MS // 128            # 4 m-chunks of 128
NG = N // 512             # 8 n-groups of 512
NCH = N // 128            # 32 n-chunks of 128
EPS = 1e-8
F32 = mybir.dt.float32
F32R = mybir.dt.float32r
BF16 = mybir.dt.bfloat16
ALU = mybir.AluOpType
AX = mybir.AxisListType

_CACHE = {}
LAST_RESULT = None


def _gram_prep(nc, smp, pwp, punp, consts, b_sb, tg):
    """Precompute inv_p, B_sl and the W-chain from a Gram matrix (SBUF)."""
    ident32_r, masksl_r, eye_r = consts

    bd = smp.tile([R, R], F32R, tag=f"bd{tg}", name="bd", bufs=1)
    nc.vector.tensor_tensor(out=bd[:], in0=b_sb[:], in1=eye_r, op=ALU.mult)
    d_p = smp.tile([R, 1], F32, tag=f"dp{tg}", name="d_p", bufs=1)
    inv_p = smp.tile([R, 1], F32, tag=f"ip{tg}", name="inv_p", bufs=1)
    nc.vector.tensor_reduce(d_p[:], bd[:], axis=AX.X, op=ALU.add)
    nc.vector.tensor_scalar_add(inv_p[:], d_p[:], EPS)
    nc.vector.reciprocal(inv_p[:], inv_p[:])
    invb = bass.AP(inv_p[:].tensor, inv_p[:].offset, [inv_p[:].ap[0], [0, R]])

    bsl = smp.tile([R, R], F32R, tag=f"bsl{tg}", name="bsl", bufs=1)
    nc.vector.tensor_tensor(out=bsl[:], in0=b_sb[:], in1=masksl_r,
                            op=ALU.mult)
    vw = smp.tile([R, R], F32R, tag=f"vw{tg}", name="vw", bufs=1)  # V = W^T
    nc.vector.tensor_tensor(out=vw[:], in0=bsl[:], in1=invb, op=ALU.mult)

    # transpose W^T -> W (f32r out must live in the f32r pun slot)
    pwt = punp.tile([128, MC, R], F32R, tag="pun", name="pwt")
    nc.tensor.transpose(pwt[:R, 0, :], vw[:], ident32_r)
    w1 = smp.tile([R, R], F32R, tag=f"w1{tg}", name="w1", bufs=1)
    nc.scalar.copy(w1[:], pwt[:R, 0, :])

    def _mm_small(lhsT, rhs, tagn):
        p = pwp.tile([R, R], F32, tag="pw", name="pmm")
        nc.tensor.matmul(p[:], lhsT=lhsT[:], rhs=rhs[:], start=True,
                         stop=True)
        s = smp.tile([R, R], F32R, tag=f"{tagn}{tg}", name=tagn, bufs=1)
        nc.scalar.copy(s[:], p[:])
        return s

    w2 = _mm_small(vw, w1, "w2")     # W^T.T @ W = W@W
    w2t = _mm_small(w1, vw, "w2t")   # W.T @ W^T = (W@W)^T
    w4 = _mm_small(w2t, w2, "w4")
    w4t = _mm_small(w2, w2t, "w4t")
    w8 = _mm_small(w4t, w4, "w8")
    w8t = _mm_small(w4, w4t, "w8t")
    w16 = _mm_small(w8t, w8, "w16")
    return {"bsl": bsl, "inv_p": inv_p, "chain": (w1, w2, w4, w8, w16)}


def _apply_solve(nc, smp, zsb, punp, zps, consts, prep, at_sb, xT_sb, out32,
                 outb16, tg):
    """sT = B_sl^T @ xT; z = P^T D'^{-1} (at - sT); back-transpose."""
    ident32_r, _, _ = consts
    bsl, inv_p = prep["bsl"], prep["inv_p"]

    ps = zps("s")
    nc.tensor.matmul(ps[:], lhsT=bsl[:], rhs=xT_sb[:], start=True, stop=True)
    z = zsb.tile([R, MS], F32R, tag=f"z{tg}", name="z0")
    nc.vector.scalar_tensor_tensor(out=z[:], in0=ps[:], scalar=-1.0,
                                   in1=at_sb[:], op0=ALU.mult, op1=ALU.add)
    nc.vector.tensor_scalar(out=z[:], in0=z[:], scalar1=inv_p[:],
                            scalar2=None, op0=ALU.mult)

    for wk, sign in zip(prep["chain"], (-1.0, 1.0, 1.0, 1.0, 1.0)):
        pz = zps("z")
        nc.tensor.matmul(pz[:], lhsT=wk[:], rhs=z[:], start=True, stop=True)
        zn = zsb.tile([R, MS], F32R, tag=f"z{tg}", name="zn")
        nc.vector.scalar_tensor_tensor(out=zn[:], in0=pz[:], scalar=sign,
                                       in1=z[:], op0=ALU.mult, op1=ALU.add)
        z = zn

    pun = punp.tile([128, MC, R], F32R, tag="pun", name="pun")
    for i in range(MC):
        nc.tensor.transpose(pun[:, i], z[:, i * 128:(i + 1) * 128],
                            ident32_r)
    nc.scalar.copy(out32[:], pun[:])
    if outb16 is not None:
        nc.vector.tensor_copy(outb16[:], pun[:])


def _build():
    nc = bacc.Bacc("TRN2", target_bir_lowering=False, debug=False,
                   num_devices=NCORES)

    x_my = nc.dram_tensor("x_my", [B, MS, N], F32, kind="ExternalInput").ap()
    u_my = nc.dram_tensor("u_my", [B, MS, R], F32, kind="ExternalInput").ap()
    v_full = nc.dram_tensor("v_full", [B, N, R], F32,
                            kind="ExternalInput").ap()
    v_my = nc.dram_tensor("v_my", [B, MS, R], F32, kind="ExternalInput").ap()
    u_out = nc.dram_tensor("u_out", [B, MS, R], F32,
                           kind="ExternalOutput").ap()
    v_out = nc.dram_tensor("v_out", [B, MS, R], F32,
                           kind="ExternalOutput").ap()

    rs_ins = [nc.dram_tensor(f"rs_in_{b}", [NCORES * R, 512 + R], F32)
              for b in range(B)]
    rs_outs = [nc.dram_tensor(f"rs_out_{b}", [R, 512 + R], F32)
               for b in range(B)]

    with tile.TileContext(nc) as tc, ExitStack() as ctx:
        const = ctx.enter_context(tc.tile_pool(name="const", bufs=1))
        xbp = ctx.enter_context(tc.tile_pool(name="xbp", bufs=1))
        xgp = ctx.enter_context(tc.tile_pool(name="xgp", bufs=3))
        xtp = ctx.enter_context(tc.tile_pool(name="xtp", bufs=4))
        vp = ctx.enter_context(tc.tile_pool(name="vp", bufs=2))
        smp = ctx.enter_context(tc.tile_pool(name="smp", bufs=2))
        zsb = ctx.enter_context(tc.tile_pool(name="zsb", bufs=3))
        a2sp = ctx.enter_context(tc.tile_pool(name="a2sp", bufs=2))
        # PSUM banks: ppt 2 + pa1 1 + pzu 1 + pw 1 + pun 1 + pa2 1 + psv 1 = 8
        ppt = ctx.enter_context(tc.tile_pool(name="ppt", bufs=2,
                                             space="PSUM"))
        pa1p = ctx.enter_context(tc.tile_pool(name="pa1", bufs=1,
                                              space="PSUM"))
        pzup = ctx.enter_context(tc.tile_pool(name="pzu", bufs=1,
                                              space="PSUM"))
        pwp = ctx.enter_context(tc.tile_pool(name="pw", bufs=1,
                                             space="PSUM"))
        punp = ctx.enter_context(tc.tile_pool(name="pun", bufs=1,
                                              space="PSUM"))
        pa2p = ctx.enter_context(tc.tile_pool(name="pa2", bufs=1,
                                              space="PSUM"))
        psvp = ctx.enter_context(tc.tile_pool(name="psv", bufs=1,
                                              space="PSUM"))

        ident128_b = const.tile([128, 128], BF16)
        make_identity(nc, ident128_b)
        ident128_f = const.tile([128, 128], F32)
        make_identity(nc, ident128_f)
        ident32_f = const.tile([R, R], F32)
        make_identity(nc, ident32_f)
        masksl_f = const.tile([R, R], F32)
        make_lower_triangular(nc, masksl_f, val=1.0, diag=False)
        ident32_r = const.tile([R, R], F32R)
        nc.vector.tensor_copy(ident32_r[:], ident32_f[:])
        masksl_r = const.tile([R, R], F32R)
        nc.vector.tensor_copy(masksl_r[:], masksl_f[:])
        consts = (ident32_r[:], masksl_r[:], ident32_r[:])

        def zps(nm):
            return pzup.tile([R, MS], F32, tag="zu", name=nm)

        def zps_v(nm):
            return psvp.tile([R, MS], F32, tag="zv", name=nm)

        # ---------- hoisted: v loads + casts; preps spread over phase1(0) --
        vbs = []
        u_preps = {}
        for b in range(B):
            v32 = vp.tile([128, NCH, R], F32, tag="v32", name="v32",
                          bufs=4)
            nc.sync.dma_start(v32[:],
                              v_full[b].rearrange("(c p) r -> p c r", p=128))
            vb = vp.tile([128, NCH, R], BF16, tag=f"vb{b}", name="vb",
                         bufs=1)
            nc.vector.tensor_copy(vb[:], v32[:])
            vbs.append(vb)

        def emit_prep(b):
            pb1 = zps("pb1")
            for j in range(NCH):
                nc.tensor.matmul(pb1[:, :R], lhsT=vbs[b][:, j, :],
                                 rhs=vbs[b][:, j, :], start=(j == 0),
                                 stop=(j == NCH - 1), skip_group_check=True)
            b1_sb = smp.tile([R, R], F32R, tag=f"b1s{b}", name="b1_sb",
                             bufs=1)
            nc.scalar.copy(b1_sb[:], pb1[:, :R])
            u_preps[b] = _gram_prep(nc, smp, pwp, punp, consts, b1_sb,
                                    f"u{b}")

        state = {}

        def emit_solve_v(b):
            st = state[b]
            a2t = smp.tile([R, MS], F32R, tag="a2t", name="a2t")
            nc.sync.dma_start(a2t[:],
                              rs_outs[b].ap()[:, 0:512].bitcast(F32R))
            b2_sb = smp.tile([R, R], F32R, tag="b2s", name="b2_sb")
            nc.sync.dma_start(b2_sb[:],
                              rs_outs[b].ap()[:, 512:512 + R].bitcast(F32R))
            at2 = smp.tile([R, MS], F32R, tag="at2", name="at2")
            nc.vector.tensor_scalar_add(at2[:], a2t[:], EPS)
            prep = _gram_prep(nc, smp, pwp, punp, consts, b2_sb, f"v{b}")
            vn32 = smp.tile([128, MC, R], F32, tag="vn32", name="vn32")
            _apply_solve(nc, smp, zsb, punp, zps_v, consts, prep, at2,
                         st["vT"], vn32, None, "v")
            nc.sync.dma_start(v_out[b].rearrange("(i p) r -> p i r", p=128),
                              vn32[:])

        p2state = {}

        def emit_phase2_part(bp, part):
            # one i-step quartet of col-tiled a2T MMs for batch bp
            gp, i = part // MC, part % MC
            unb_p, xb_p = p2state[bp]
            if i == 0:
                p2state[f"pa2_{bp}_{gp}"] = pa2p.tile(
                    [128, MS], F32, tag="pa2", name="pa2")
            pa2 = p2state[f"pa2_{bp}_{gp}"]
            for p in range(4):
                g2 = gp * 4 + p
                nc.tensor.matmul(
                    pa2[p * R:(p + 1) * R, :], lhsT=unb_p[:, i, :],
                    rhs=xb_p[:, i, g2 * 512:(g2 + 1) * 512],
                    start=(i == 0), stop=(i == MC - 1),
                    tile_position=(0, p * R), skip_group_check=True)
            if i == MC - 1:
                a2st = a2sp.tile([128, MS], F32, tag="a2st", name="a2st")
                nc.scalar.copy(a2st[:], pa2[:])
                nc.sync.dma_start(
                    rs_ins[bp].ap()[ds(gp * 4 * R, 4 * R), 0:512], a2st[:])

        def finish_phase2(bp):
            unb_p, _ = p2state[bp]
            pb2 = pwp.tile([R, R], F32, tag="pw", name="pb2")
            for i in range(MC):
                nc.tensor.matmul(pb2[:], lhsT=unb_p[:, i, :],
                                 rhs=unb_p[:, i, :], start=(i == 0),
                                 stop=(i == MC - 1), skip_group_check=True)
            b2st = a2sp.tile([R, R], F32, tag="b2st", name="b2st")
            nc.scalar.copy(b2st[:], pb2[:])
            for c in range(NCORES):
                nc.sync.dma_start(
                    rs_ins[bp].ap()[ds(c * R, R), 512:512 + R], b2st[:])
            nc.gpsimd.collective_compute(
                "ReduceScatter", ALU.add,
                replica_groups=[list(range(NCORES))],
                ins=[rs_ins[bp].ap()], outs=[rs_outs[bp].ap()])

        for b in range(B):
            bi = b % 2
            # ---------------- per-batch loads + uT/vT ----------------
            u32 = vp.tile([128, MC, R], F32, tag="u32", name="u32")
            nc.sync.dma_start(u32[:],
                              u_my[b].rearrange("(i p) r -> p i r", p=128))
            vm32 = vp.tile([128, MC, R], F32, tag="vm32", name="vm32")
            nc.sync.dma_start(vm32[:],
                              v_my[b].rearrange("(i p) r -> p i r", p=128))

            put = zps("put")
            for i in range(MC):
                nc.tensor.transpose(put[:, i * 128:(i + 1) * 128],
                                    u32[:, i, :], ident128_f[:])
            uT = smp.tile([R, MS], F32R, tag="uT", name="uT")
            nc.scalar.copy(uT[:], put[:])
            pvt = zps("pvt")
            for i in range(MC):
                nc.tensor.transpose(pvt[:, i * 128:(i + 1) * 128],
                                    vm32[:, i, :], ident128_f[:])
            vT = smp.tile([R, MS], F32R, tag="vT", name="vT", bufs=3)
            nc.scalar.copy(vT[:], pvt[:])

            # ---------------- phase 1: stream x ----------------
            xb_t = xbp.tile([128, MC, N], BF16, tag=f"xb{bi}", name="xb")
            pa1 = pa1p.tile([R, MS], F32, tag="pa1", name="pa1")
            x_re = x_my[b].rearrange("(i p) n -> p i n", p=128)
            for g in range(NG):
                xg = xgp.tile([128, MC, 512], F32, tag="xg", name="xg")
                nc.sync.dma_start(xg[:], x_re[:, :, g * 512:(g + 1) * 512])
                nc.vector.tensor_copy(xb_t[:, :, g * 512:(g + 1) * 512],
                                      xg[:])
                for j2 in range(4):
                    j = 4 * g + j2
                    pt = ppt.tile([128, MC, 128], BF16, tag="pt", name="pt")
                    for i in range(MC):
                        nc.tensor.transpose(
                            pt[:, i], xb_t[:, i, j * 128:(j + 1) * 128],
                            ident128_b[:])
                    xt = xtp.tile([128, MC, 128], BF16, tag="xt", name="xt")
                    nc.scalar.copy(xt[:], pt[:])
                    nc.tensor.matmul(pa1[:], lhsT=vbs[b][:, j, :],
                                     rhs=xt.rearrange("p a b -> p (a b)"),
                                     start=(j == 0), stop=(j == NCH - 1),
                                     skip_group_check=True)
                if b == 0 and g < B:
                    emit_prep(g)
                if b >= 1:
                    emit_phase2_part(b - 1, g)
            if b >= 1:
                finish_phase2(b - 1)
                if b >= 3:
                    emit_solve_v(b - 3)

            # ---------------- u solve ----------------
            at1 = smp.tile([R, MS], F32R, tag="at1", name="at1")
            nc.vector.tensor_scalar_add(at1[:], pa1[:], EPS)
            un32 = smp.tile([128, MC, R], F32, tag="un32", name="un32")
            unb = smp.tile([128, MC, R], BF16, tag="unb", name="unb")
            _apply_solve(nc, smp, zsb, punp, zps, consts, u_preps[b], at1,
                         uT, un32, unb, "u")
            nc.sync.dma_start(u_out[b].rearrange("(i p) r -> p i r", p=128),
                              un32[:])

            p2state[b] = (unb, xb_t)
            state[b] = {"vT": vT}

        # last batch: dense phase 2 + RS, then remaining v-solves
        for part in range(NG):
            emit_phase2_part(B - 1, part)
        finish_phase2(B - 1)
        emit_solve_v(B - 3)
        emit_solve_v(B - 2)
        emit_solve_v(B - 1)

    nc.compile()
    return nc


def kernel(x, u, v):
    global LAST_RESULT
    if "nc" not in _CACHE:
        _CACHE["nc"] = _build()
    nc = _CACHE["nc"]

    x = np.ascontiguousarray(x, dtype=np.float32)
    u = np.ascontiguousarray(u, dtype=np.float32)
    v = np.ascontiguousarray(v, dtype=np.float32)

    in_maps = []
    for c in range(NCORES):
        sl = slice(c * MS, (c + 1) * MS)
        in_maps.append({
            "x_my": np.ascontiguousarray(x[:, sl, :]),
            "u_my": np.ascontiguousarray(u[:, sl, :]),
            "v_full": v,
            "v_my": np.ascontiguousarray(v[:, sl, :]),
        })

    res = run_bass_kernel_spmd(nc, in_maps, list(range(NCORES)),
                               trace=os.environ.get("KBENCH_TRACE") == "1")
    LAST_RESULT = res
    u_new = np.concatenate([res.results[c]["u_out"] for c in range(NCORES)],
                           axis=1)
    v_new = np.concatenate([res.results[c]["v_out"] for c in range(NCORES)],
                           axis=1)
    return (u_new, v_new)
